# revision 26
# baseline (speedup 1.0000x reference)
"""nn_Attention_6373731467473 — linear attention w/ head expansion + LePE.

Full-input contract: kernel(**inputs) takes unsharded inputs, returns full
output. Data-parallel over batch: 8 batch elements -> 8 NeuronCores, no
collectives. Per core, everything runs in bf16 matmuls with fp32 PSUM
accumulation (tolerance is 2e-2 relative).

Pipeline per core (batch element b):
  P1: stream xT tiles; qT = (x @ w_q)^T   [qcol, n] resident SBUF
      k,v row-major per 128-row chunk; softmax(k) over head_dim;
      ktv[h] = softmax(k)_h^T @ v_h accumulated over n.
  P2: assemble block-diag expanded-ktv lhsT tiles (DMA SBUF->SBUF),
      scale 1/sqrt(64) folded in.
  P3: per 1024-col tile: attn^T chunks via block-diag matmuls on qT /
      rolled qT; LePE depthwise 3x3 conv added in-place on DVE as 9
      per-partition-scaled shifted accumulations; proj matmuls into y.

Host prep: transpose+cast x to bf16 (768, 4096) per batch, split w_kv,
fold b_lepe @ w_proj + b_proj into one bias, LePE taps as (1536, 9).
"""

import hashlib
import os
import pickle
from contextlib import ExitStack

import numpy as np

B, N, DIM = 8, 4096, 768
HEADS, HD = 12, 64
EXP = 2
EDIM = EXP * DIM  # 1536
CC = DIM // 128   # 6 contraction chunks
EC = EDIM // 128  # 12 expanded chunks
NT1 = 512         # phase-1 n-tile
NT3 = 1024        # phase-3 n-tile
SCALE = HD ** -0.5

_CACHE_DIR = os.environ.get("BASS_NEFF_DISK_CACHE", "/root/.cache/bass_neff_cache")


def _install_cc_cache():
    """Disk-cache the HLO->NEFF compile (walrus is the slow step)."""
    try:
        import libneuronxla
        from concourse import bass2jax

        bass2jax.install_neuronx_cc_hook()
        inner = bass2jax.neuronx_cc_hook

        def cached_cc(code, code_format, platform_version, file_prefix):
            try:
                key = hashlib.sha256(
                    b"v1|" + bytes(code) + b"|" + bytes(code_format)
                ).hexdigest()
                path = os.path.join(_CACHE_DIR, key + ".pkl")
                if os.path.exists(path):
                    with open(path, "rb") as f:
                        return pickle.load(f)
            except Exception:
                path = None
            r = inner(code, code_format, platform_version, file_prefix)
            if path is not None:
                try:
                    os.makedirs(_CACHE_DIR, exist_ok=True)
                    tmp = path + f".tmp{os.getpid()}"
                    with open(tmp, "wb") as f:
                        pickle.dump(r, f)
                    os.replace(tmp, path)
                except Exception:
                    pass
            return r

        libneuronxla.neuronx_cc = cached_cc
    except Exception:
        pass


def _build_nc(consts):
    import concourse.bacc as bacc
    import concourse.mybir as mybir
    import concourse.tile as tile

    f32 = mybir.dt.float32
    bf16 = mybir.dt.bfloat16
    AX = mybir.AxisListType
    OP = mybir.AluOpType
    AF = mybir.ActivationFunctionType

    nc = bacc.Bacc("TRN2", target_bir_lowering=False, debug=False, num_devices=B)

    xt_d = nc.dram_tensor("xt", [DIM, N], bf16, kind="ExternalInput").ap()
    wq_d = nc.inline_tensor(consts["wq"], "wq").ap()
    wkv_d = nc.inline_tensor(consts["wkv"], "wkv").ap()
    wp_d = nc.inline_tensor(consts["wp"], "wp").ap()
    taps_d = nc.inline_tensor(consts["taps"], "taps").ap()
    bias_d = nc.inline_tensor(consts["bias"], "bias").ap()
    y_d = nc.dram_tensor("y", [N, DIM], bf16, kind="ExternalOutput").ap()

    with tile.TileContext(nc) as tc, ExitStack() as ctx:
        persist = ctx.enter_context(tc.tile_pool(name="persist", bufs=1))
        qT = persist.tile([128, CC, N], bf16)           # q^T, chunk-major
        wp_sb = persist.tile([128, EC, DIM], bf16)
        taps_sb = persist.tile([128, EC, 9], f32)
        bias_sb = persist.tile([128, DIM], f32)
        ektv_sb = persist.tile([128, EC, 128], bf16)    # block-diag lhsT per pair
        ktv_sb = persist.tile([64, HEADS * HD], bf16)   # scaled bf16 ktv

        nc.sync.dma_start(wp_sb, wp_d.rearrange("(t p) m -> p t m", p=128))
        nc.sync.dma_start(taps_sb, taps_d.rearrange("(t p) s -> p t s", p=128))
        nc.sync.dma_start(bias_sb, bias_d)

        # ---------------- Phase 1: qT, k/v, softmax, ktv ----------------
        with tc.tile_pool(name="p1", bufs=3) as p1, \
             tc.tile_pool(name="p1w", bufs=1) as p1w, \
             tc.tile_pool(name="ps_q", bufs=2, space="PSUM") as ps_q, \
             tc.tile_pool(name="ps_kv", bufs=1, space="PSUM") as ps_kv, \
             tc.tile_pool(name="ps_ktv", bufs=1, space="PSUM") as ps_ktv:
            wq_sb = p1w.tile([128, CC, DIM], bf16)
            wkv_sb = p1w.tile([128, CC, 2 * DIM], bf16)
            nc.sync.dma_start(wq_sb, wq_d.rearrange("(t p) m -> p t m", p=128))
            nc.sync.dma_start(wkv_sb, wkv_d.rearrange("(t p) m -> p t m", p=128))

            xt_r = xt_d.rearrange("(c p) n -> p c n", p=128)
            # single PSUM accumulator for ktv across the whole n loop; each
            # bank's first matmul carries start=True (clears has_written once)
            ktv_ps = ps_ktv.tile([64, HEADS * HD], f32)
            n_tiles1 = N // NT1
            subs1 = NT1 // 128
            for it in range(n_tiles1):
                n0 = it * NT1
                xt_sb = p1.tile([128, CC, NT1], bf16, tag="xt")
                nc.sync.dma_start(xt_sb, xt_r[:, :, n0:n0 + NT1])

                # qT chunks
                for t in range(CC):
                    q_ps = ps_q.tile([128, NT1], f32, tag="q")
                    for cc in range(CC):
                        nc.tensor.matmul(
                            q_ps, wq_sb[:, cc, 128 * t:128 * (t + 1)],
                            xt_sb[:, cc, :],
                            start=(cc == 0), stop=(cc == CC - 1),
                        )
                    nc.scalar.copy(out=qT[:, t, n0:n0 + NT1], in_=q_ps)

                # k/v rows (joint 1536-wide matmul), softmax, ktv accumulation.
                # Pass 1 computes exp(k)/v and per-head sums for all 4 subs;
                # one batched fast-reciprocal; pass 2 normalizes and runs ktv.
                exp_sb = p1.tile([128, subs1, HEADS, HD], bf16, tag="exp")
                v_bf = p1.tile([128, subs1, DIM], bf16, tag="vb")
                ssum = p1.tile([128, subs1, HEADS], f32, tag="ssum")
                rec = p1.tile([128, subs1, HEADS], f32, tag="rec")
                for sub in range(subs1):
                    kv_ps = ps_kv.tile([128, 2 * DIM], f32, tag="kv")
                    for cc in range(CC):
                        lhs = xt_sb[:, cc, 128 * sub:128 * (sub + 1)]
                        st, sp = (cc == 0), (cc == CC - 1)
                        for blk in range(3):
                            nc.tensor.matmul(
                                kv_ps[:, 512 * blk:512 * (blk + 1)], lhs,
                                wkv_sb[:, cc, 512 * blk:512 * (blk + 1)],
                                start=st, stop=sp)
                    nc.scalar.activation(
                        out=exp_sb[:, sub].rearrange("p h d -> p (h d)"),
                        in_=kv_ps[:, 0:DIM], func=AF.Exp)
                    nc.vector.reduce_sum(ssum[:, sub], exp_sb[:, sub], axis=AX.X)
                    nc.scalar.copy(out=v_bf[:, sub], in_=kv_ps[:, DIM:2 * DIM])
                nc.vector.reciprocal_approx_fast(
                    out=rec.rearrange("p s h -> p (s h)"),
                    in_=ssum.rearrange("p s h -> p (s h)"))
                for sub in range(subs1):
                    ks_bf = p1.tile([128, HEADS, HD], bf16, tag="ks")
                    nc.vector.tensor_tensor(
                        ks_bf, exp_sb[:, sub],
                        rec[:, sub, :, None].broadcast_to([128, HEADS, HD]),
                        OP.mult)
                    first = (it == 0 and sub == 0)
                    last = (it == n_tiles1 - 1 and sub == subs1 - 1)
                    for h in range(HEADS):
                        nc.tensor.matmul(
                            ktv_ps[:, HD * h:HD * (h + 1)],
                            ks_bf[:, h, :], v_bf[:, sub, HD * h:HD * (h + 1)],
                            start=(first and h % 8 == 0),
                            stop=(last and h in (7, 11)),
                            skip_group_check=True,
                        )

            # scale into bf16 (attention scale folded into ektv)
            nc.scalar.mul(out=ktv_sb, in_=ktv_ps, mul=SCALE)

        # ---------------- Phase 2: block-diag expanded ktv ----------------
        nc.vector.memset(ektv_sb, 0.0)
        for p in range(6):  # non-rolled pairs: heads 2p, 2p+1
            h0, h1 = 2 * p, 2 * p + 1
            nc.sync.dma_start(ektv_sb[0:64, p, 0:64],
                              ktv_sb[:, HD * h0:HD * (h0 + 1)])
            nc.sync.dma_start(ektv_sb[64:128, p, 64:128],
                              ktv_sb[:, HD * h1:HD * (h1 + 1)])
        for r in range(6):  # rolled pairs p=6+r: expanded heads 12+2r, 13+2r
            p = 6 + r
            h, h2 = 2 * r, 2 * r + 1
            h3 = (h2 + 1) % HEADS
            nc.sync.dma_start(ektv_sb[0:64, p, 0:32],
                              ktv_sb[:, HD * h + 32:HD * (h + 1)])
            nc.sync.dma_start(ektv_sb[0:64, p, 32:64],
                              ktv_sb[:, HD * h2:HD * h2 + 32])
            nc.sync.dma_start(ektv_sb[64:128, p, 64:96],
                              ktv_sb[:, HD * h2 + 32:HD * (h2 + 1)])
            nc.sync.dma_start(ektv_sb[64:128, p, 96:128],
                              ktv_sb[:, HD * h3:HD * h3 + 32])

        # ---------------- Phase 3: attn + LePE + proj ----------------
        # All taps on DVE, in 3 independent chains grouped by dx so the
        # in-place RAW chains interleave (pipe-drain overlap): dx=0 taps
        # accumulate onto mt (attn already there); dx=+1 onto mtB (seeded by
        # its dy=0 tap, all write x 0:63); dx=-1 onto mtC (x 1:64). Two
        # range-limited merges fold mtB/mtC into mt.
        CHAIN_A = [(0, 0), (-1, 0), (1, 0)]
        CHAIN_B = [(0, 1), (-1, 1), (1, 1)]
        CHAIN_C = [(0, -1), (-1, -1), (1, -1)]
        with tc.tile_pool(name="p3", bufs=2) as p3, \
             tc.tile_pool(name="ps_at", bufs=2, space="PSUM") as ps_at, \
             tc.tile_pool(name="ps_y", bufs=2, space="PSUM") as ps_y:
            for it in range(N // NT3):
                n0 = it * NT3
                rows = NT3 // 64          # image rows in this tile
                y0 = n0 // 64             # first global image row
                # rolled-q stream tile with 64-halo on both sides
                a = max(0, n0 - 64)
                b = min(N, n0 + NT3 + 64)
                off = a - (n0 - 64)
                qtr = p3.tile([128, CC, NT3 + 128], bf16, tag="qtr")
                for t in range(CC):
                    nc.scalar.dma_start(qtr[0:96, t, off:off + (b - a)],
                                        qT[32:128, t, a:b])
                    nc.scalar.dma_start(qtr[96:128, t, off:off + (b - a)],
                                        qT[0:32, (t + 1) % CC, a:b])

                mt = p3.tile([128, EC, NT3], bf16, tag="mt")
                for p in range(EC):
                    mtB = p3.tile([128, NT3], bf16, tag="mtB")
                    mtC = p3.tile([128, NT3], bf16, tag="mtC")
                    for half in range(NT3 // 512):
                        at_ps = ps_at.tile([128, 512], f32, tag="at")
                        if p < 6:
                            rhs = qT[:, p, n0 + 512 * half:n0 + 512 * (half + 1)]
                        else:
                            rhs = qtr[:, p - 6,
                                      64 + 512 * half:64 + 512 * (half + 1)]
                        nc.tensor.matmul(at_ps, ektv_sb[:, p, :], rhs,
                                         start=True, stop=True)
                        nc.scalar.copy(out=mt[:, p, 512 * half:512 * (half + 1)],
                                       in_=at_ps)

                    out3 = {
                        0: mt[:, p, :].rearrange("p (y x) -> p y x", x=64),
                        1: mtB.rearrange("p (y x) -> p y x", x=64),
                        -1: mtC.rearrange("p (y x) -> p y x", x=64),
                    }
                    if p < 6:
                        src3 = qT[:, p, :].rearrange("p (y x) -> p y x", x=64)
                    else:
                        src3 = qtr[:, p - 6, :].rearrange("p (y x) -> p y x", x=64)
                    # interleave the three chains so DVE pipe-drains overlap
                    for (dy, dx) in [c[i] for i in range(3)
                                     for c in (CHAIN_A, CHAIN_B, CHAIN_C)]:
                        r0 = max(0, -(y0 + dy))
                        r1 = rows - max(0, y0 + rows - 1 + dy - 63)
                        if dx == 1:
                            xo, xi = (0, 63), (1, 64)
                        elif dx == -1:
                            xo, xi = (1, 64), (0, 63)
                        else:
                            xo, xi = (0, 64), (0, 64)
                        if p < 6:
                            s0 = y0 + r0 + dy
                            s1 = y0 + r1 + dy
                        else:
                            s0 = r0 + dy + 1
                            s1 = r1 + dy + 1
                        widx = (dy + 1) * 3 + (dx + 1)
                        w_ap = taps_sb[:, p, widx:widx + 1]
                        i_ap = src3[:, s0:s1, xi[0]:xi[1]]
                        o_ap = out3[dx][:, r0:r1, xo[0]:xo[1]]
                        if dy == 0 and dx != 0:
                            # chain seed: overwrite (full row range for dy=0)
                            nc.vector.tensor_scalar(
                                out=o_ap, in0=i_ap, scalar1=w_ap,
                                scalar2=None, op0=OP.mult)
                        else:
                            nc.vector.scalar_tensor_tensor(
                                out=o_ap, in0=i_ap, scalar=w_ap,
                                in1=o_ap, op0=OP.mult, op1=OP.add)
                    m3 = mt[:, p, :].rearrange("p (y x) -> p y x", x=64)
                    b3 = mtB.rearrange("p (y x) -> p y x", x=64)
                    c3 = mtC.rearrange("p (y x) -> p y x", x=64)
                    nc.vector.tensor_tensor(
                        m3[:, :, 0:63], m3[:, :, 0:63], b3[:, :, 0:63], OP.add)
                    nc.vector.tensor_tensor(
                        m3[:, :, 1:64], m3[:, :, 1:64], c3[:, :, 1:64], OP.add)

                # proj
                for sub in range(NT3 // 128):
                    y_ps = ps_y.tile([128, DIM], f32, tag="y")
                    for e in range(EC):
                        lhs = mt[:, e, 128 * sub:128 * (sub + 1)]
                        st, sp = (e == 0), (e == EC - 1)
                        nc.tensor.matmul(y_ps[:, 0:512], lhs, wp_sb[:, e, 0:512],
                                         start=st, stop=sp)
                        nc.tensor.matmul(y_ps[:, 512:768], lhs, wp_sb[:, e, 512:768],
                                         start=st, stop=sp)
                    y_sb = p3.tile([128, DIM], bf16, tag="ysb")
                    nc.vector.tensor_tensor(y_sb, y_ps, bias_sb, OP.add)
                    nc.sync.dma_start(
                        y_d[n0 + 128 * sub:n0 + 128 * (sub + 1), :], y_sb)

    nc.compile()
    return nc


def _run(nc, xt_dev):
    """Execute the prebuilt Bass module on B cores via PJRT/shard_map.

    Like bass2jax.run_bass_via_pjrt, but output buffers are created
    device-side (no host->device upload of donated zeros) and the input
    is already device-resident.
    """
    import jax
    import jax.numpy as jnp
    import concourse.mybir as mybir
    from concourse.bass2jax import _bass_exec_p, partition_id_tensor
    from jax.experimental.shard_map import shard_map
    from jax.sharding import Mesh, NamedSharding, PartitionSpec

    partition_name = (
        nc.partition_id_tensor.name if nc.partition_id_tensor else None)
    in_names, out_names, out_avals = [], [], []
    for alloc in nc.m.functions[0].allocations:
        if not isinstance(alloc, mybir.MemoryLocationSet):
            continue
        name = alloc.memorylocations[0].name
        if alloc.kind == "ExternalInput":
            if name != partition_name:
                in_names.append(name)
        elif alloc.kind == "ExternalOutput":
            out_names.append(name)
            out_avals.append(jax.core.ShapedArray(
                tuple(alloc.tensor_shape), mybir.dt.np(alloc.dtype)))
    assert in_names == ["xt"] and out_names == ["y"], (in_names, out_names)
    n_params = len(in_names)
    n_outs = len(out_avals)
    donate = tuple(range(n_params, n_params + n_outs))
    all_names = in_names + out_names
    if partition_name is not None:
        all_names = all_names + [partition_name]

    def _body(*args):
        operands = list(args)
        if partition_name is not None:
            operands.append(partition_id_tensor())
        outs = _bass_exec_p.bind(
            *operands,
            out_avals=tuple(out_avals),
            in_names=tuple(all_names),
            out_names=tuple(out_names),
            lowering_input_output_aliases=(),
            sim_require_finite=True,
            sim_require_nnan=True,
            nc=nc,
        )
        return tuple(outs)

    mesh, shard, pspec = _mesh_shard()
    if _sess.get("exec_nc") is not nc:
        sharded = jax.jit(
            shard_map(_body, mesh=mesh, in_specs=(pspec,) * (n_params + n_outs),
                      out_specs=(pspec,) * n_outs, check_rep=False),
            donate_argnums=donate, keep_unused=True)
        _sess["exec"] = sharded.lower(
            jax.ShapeDtypeStruct((B * DIM, N), xt_dev.dtype),
            *[jax.ShapeDtypeStruct((B * a.shape[0], *a.shape[1:]), a.dtype)
              for a in out_avals],
        ).compile()
        _sess["zeros_jit"] = [
            jax.jit(lambda a=a: jnp.zeros((B * a.shape[0], *a.shape[1:]),
                                          a.dtype), out_shardings=shard)
            for a in out_avals
        ]
        _sess["exec_nc"] = nc
    zeros_dev = [zj() for zj in _sess["zeros_jit"]]
    outs = _sess["exec"](xt_dev, *zeros_dev)
    return np.asarray(outs[0]).reshape(B, N, DIM)


def _mesh_shard():
    import jax
    from jax.experimental.shard_map import shard_map  # noqa: F401
    from jax.sharding import Mesh, NamedSharding, PartitionSpec

    devices = jax.devices()[:B]
    mesh = Mesh(np.asarray(devices), ("core",))
    pspec = PartitionSpec("core")
    return mesh, NamedSharding(mesh, pspec), pspec


_sess = {}


def kernel(x, w_q, w_kv, w_proj, b_proj, w_lepe, b_lepe):
    import ml_dtypes

    _install_cc_cache()

    bf = ml_dtypes.bfloat16
    x = np.asarray(x, np.float32)
    w_q = np.asarray(w_q, np.float32)
    w_kv = np.asarray(w_kv, np.float32)
    w_proj = np.asarray(w_proj, np.float32)
    b_proj = np.asarray(b_proj, np.float32)
    w_lepe = np.asarray(w_lepe, np.float32)
    b_lepe = np.asarray(b_lepe, np.float32)

    xt = np.ascontiguousarray(x.transpose(0, 2, 1)).astype(bf)  # (B, DIM, N)
    consts = {
        "wq": np.ascontiguousarray(w_q).astype(bf),
        "wkv": np.ascontiguousarray(w_kv).astype(bf),
        "wp": np.ascontiguousarray(w_proj).astype(bf),
        "taps": np.ascontiguousarray(w_lepe.reshape(EDIM, 9)).astype(np.float32),
        "bias": np.ascontiguousarray(np.broadcast_to(
            (b_proj.astype(np.float64)
             + b_lepe.astype(np.float64) @ w_proj.astype(np.float64)
             ).astype(np.float32), (128, DIM))),
    }
    key = hashlib.sha256(
        b"|".join(np.ascontiguousarray(v).tobytes() for v in consts.values())
    ).hexdigest()

    # overlap the (bandwidth-bound) input upload with nc build + jit wrap
    import threading

    import jax

    _, shard, _ = _mesh_shard()  # forces jax/axon init in main thread
    xin = xt.reshape(B * DIM, N)
    box = {}

    def _uploader():
        try:
            box["xdev"] = jax.device_put(xin, shard)
            jax.block_until_ready(box["xdev"])
        except Exception as e:  # fall back to host array
            box["err"] = e

    th = threading.Thread(target=_uploader)
    th.start()

    if _sess.get("key") != key:
        _sess["nc"] = _build_nc(consts)
        _sess["key"] = key

    th.join()
    xt_dev = box.get("xdev", xin)
    y = _run(_sess["nc"], xt_dev)
    return y.astype(np.float32)


# revision 29
# speedup vs baseline: 1.0962x; 1.0962x over previous
"""nn_Attention_6373731467473 — linear attention w/ head expansion + LePE.

Full-input contract: kernel(**inputs) takes unsharded inputs, returns full
output. Data-parallel over batch: 8 batch elements -> 8 NeuronCores, no
collectives. Per core, everything runs in bf16 matmuls with fp32 PSUM
accumulation (tolerance is 2e-2 relative).

Pipeline per core (batch element b):
  P1: stream xT tiles; qT = (x @ w_q)^T   [qcol, n] resident SBUF
      k,v row-major per 128-row chunk; softmax(k) over head_dim;
      ktv[h] = softmax(k)_h^T @ v_h accumulated over n.
  P2: assemble block-diag expanded-ktv lhsT tiles (DMA SBUF->SBUF),
      scale 1/sqrt(64) folded in.
  P3: per 1024-col tile: attn^T chunks via block-diag matmuls on qT /
      rolled qT; LePE depthwise 3x3 conv added in-place on DVE as 9
      per-partition-scaled shifted accumulations; proj matmuls into y.

Host prep: transpose+cast x to bf16 (768, 4096) per batch, split w_kv,
fold b_lepe @ w_proj + b_proj into one bias, LePE taps as (1536, 9).
"""

import hashlib
import os
import pickle
from contextlib import ExitStack

import numpy as np

B, N, DIM = 8, 4096, 768
HEADS, HD = 12, 64
EXP = 2
EDIM = EXP * DIM  # 1536
CC = DIM // 128   # 6 contraction chunks
EC = EDIM // 128  # 12 expanded chunks
NT1 = 512         # phase-1 n-tile
NT3 = 1024        # phase-3 n-tile
SCALE = HD ** -0.5

_CACHE_DIR = os.environ.get("BASS_NEFF_DISK_CACHE", "/root/.cache/bass_neff_cache")


def _install_cc_cache():
    """Disk-cache the HLO->NEFF compile (walrus is the slow step)."""
    try:
        import libneuronxla
        from concourse import bass2jax

        bass2jax.install_neuronx_cc_hook()
        inner = bass2jax.neuronx_cc_hook

        def cached_cc(code, code_format, platform_version, file_prefix):
            try:
                key = hashlib.sha256(
                    b"v1|" + bytes(code) + b"|" + bytes(code_format)
                ).hexdigest()
                path = os.path.join(_CACHE_DIR, key + ".pkl")
                if os.path.exists(path):
                    with open(path, "rb") as f:
                        return pickle.load(f)
            except Exception:
                path = None
            r = inner(code, code_format, platform_version, file_prefix)
            if path is not None:
                try:
                    os.makedirs(_CACHE_DIR, exist_ok=True)
                    tmp = path + f".tmp{os.getpid()}"
                    with open(tmp, "wb") as f:
                        pickle.dump(r, f)
                    os.replace(tmp, path)
                except Exception:
                    pass
            return r

        libneuronxla.neuronx_cc = cached_cc
    except Exception:
        pass


def _build_nc(consts):
    import concourse.bacc as bacc
    import concourse.mybir as mybir
    import concourse.tile as tile

    f32 = mybir.dt.float32
    bf16 = mybir.dt.bfloat16
    AX = mybir.AxisListType
    OP = mybir.AluOpType
    AF = mybir.ActivationFunctionType

    nc = bacc.Bacc("TRN2", target_bir_lowering=False, debug=False, num_devices=B)

    xt_d = nc.dram_tensor("xt", [DIM, N], bf16, kind="ExternalInput").ap()
    wq_d = nc.inline_tensor(consts["wq"], "wq").ap()
    wkv_d = nc.inline_tensor(consts["wkv"], "wkv").ap()
    wp_d = nc.inline_tensor(consts["wp"], "wp").ap()
    taps_d = nc.inline_tensor(consts["taps"], "taps").ap()
    bias_d = nc.inline_tensor(consts["bias"], "bias").ap()
    y_d = nc.dram_tensor("y", [N, DIM], bf16, kind="ExternalOutput").ap()

    with tile.TileContext(nc) as tc, ExitStack() as ctx:
        persist = ctx.enter_context(tc.tile_pool(name="persist", bufs=1))
        qT = persist.tile([128, CC, N], bf16)           # q^T, chunk-major
        wp_sb = persist.tile([128, EC, DIM], bf16)
        taps_sb = persist.tile([128, EC, 9], f32)
        bias_sb = persist.tile([128, DIM], f32)
        ektv_sb = persist.tile([128, EC, 128], bf16)    # block-diag lhsT per pair
        ktv_sb = persist.tile([64, HEADS * HD], bf16)   # scaled bf16 ktv

        nc.gpsimd.dma_start(wp_sb, wp_d.rearrange("(t p) m -> p t m", p=128))
        nc.gpsimd.dma_start(taps_sb, taps_d.rearrange("(t p) s -> p t s", p=128))
        nc.gpsimd.dma_start(bias_sb, bias_d)

        # ---------------- Phase 1: qT, k/v, softmax, ktv ----------------
        with tc.tile_pool(name="p1", bufs=3) as p1, \
             tc.tile_pool(name="p1w", bufs=1) as p1w, \
             tc.tile_pool(name="ps_q", bufs=2, space="PSUM") as ps_q, \
             tc.tile_pool(name="ps_kv", bufs=1, space="PSUM") as ps_kv, \
             tc.tile_pool(name="ps_ktv", bufs=1, space="PSUM") as ps_ktv:
            wq_sb = p1w.tile([128, CC, DIM], bf16)
            wkv_sb = p1w.tile([128, CC, 2 * DIM], bf16)
            nc.scalar.dma_start(wq_sb, wq_d.rearrange("(t p) m -> p t m", p=128))
            nc.sync.dma_start(wkv_sb, wkv_d.rearrange("(t p) m -> p t m", p=128))

            xt_r = xt_d.rearrange("(c p) n -> p c n", p=128)
            # single PSUM accumulator for ktv across the whole n loop; each
            # bank's first matmul carries start=True (clears has_written once)
            ktv_ps = ps_ktv.tile([64, HEADS * HD], f32)
            n_tiles1 = N // NT1
            subs1 = NT1 // 128
            for it in range(n_tiles1):
                n0 = it * NT1
                xt_sb = p1.tile([128, CC, NT1], bf16, tag="xt")
                nc.sync.dma_start(xt_sb, xt_r[:, :, n0:n0 + NT1])

                # qT chunks
                for t in range(CC):
                    q_ps = ps_q.tile([128, NT1], f32, tag="q")
                    for cc in range(CC):
                        nc.tensor.matmul(
                            q_ps, wq_sb[:, cc, 128 * t:128 * (t + 1)],
                            xt_sb[:, cc, :],
                            start=(cc == 0), stop=(cc == CC - 1),
                        )
                    nc.scalar.copy(out=qT[:, t, n0:n0 + NT1], in_=q_ps)

                # k/v rows (joint 1536-wide matmul), softmax, ktv accumulation.
                # Pass 1 computes exp(k)/v and per-head sums for all 4 subs;
                # one batched fast-reciprocal; pass 2 normalizes and runs ktv.
                exp_sb = p1.tile([128, subs1, HEADS, HD], bf16, tag="exp")
                v_bf = p1.tile([128, subs1, DIM], bf16, tag="vb")
                ssum = p1.tile([128, subs1, HEADS], f32, tag="ssum")
                rec = p1.tile([128, subs1, HEADS], f32, tag="rec")
                for sub in range(subs1):
                    kv_ps = ps_kv.tile([128, 2 * DIM], f32, tag="kv")
                    for cc in range(CC):
                        lhs = xt_sb[:, cc, 128 * sub:128 * (sub + 1)]
                        st, sp = (cc == 0), (cc == CC - 1)
                        for blk in range(3):
                            nc.tensor.matmul(
                                kv_ps[:, 512 * blk:512 * (blk + 1)], lhs,
                                wkv_sb[:, cc, 512 * blk:512 * (blk + 1)],
                                start=st, stop=sp)
                    nc.scalar.activation(
                        out=exp_sb[:, sub].rearrange("p h d -> p (h d)"),
                        in_=kv_ps[:, 0:DIM], func=AF.Exp)
                    nc.vector.reduce_sum(ssum[:, sub], exp_sb[:, sub], axis=AX.X)
                    nc.scalar.copy(out=v_bf[:, sub], in_=kv_ps[:, DIM:2 * DIM])
                nc.vector.reciprocal_approx_fast(
                    out=rec.rearrange("p s h -> p (s h)"),
                    in_=ssum.rearrange("p s h -> p (s h)"))
                for sub in range(subs1):
                    ks_bf = p1.tile([128, HEADS, HD], bf16, tag="ks")
                    nc.vector.tensor_tensor(
                        ks_bf, exp_sb[:, sub],
                        rec[:, sub, :, None].broadcast_to([128, HEADS, HD]),
                        OP.mult)
                    first = (it == 0 and sub == 0)
                    last = (it == n_tiles1 - 1 and sub == subs1 - 1)
                    for h in range(HEADS):
                        nc.tensor.matmul(
                            ktv_ps[:, HD * h:HD * (h + 1)],
                            ks_bf[:, h, :], v_bf[:, sub, HD * h:HD * (h + 1)],
                            start=(first and h % 8 == 0),
                            stop=(last and h in (7, 11)),
                            skip_group_check=True,
                        )

            # scale into bf16 (attention scale folded into ektv)
            nc.scalar.mul(out=ktv_sb, in_=ktv_ps, mul=SCALE)

        # ---------------- Phase 2: block-diag expanded ktv ----------------
        nc.vector.memset(ektv_sb, 0.0)
        for p in range(6):  # non-rolled pairs: heads 2p, 2p+1
            h0, h1 = 2 * p, 2 * p + 1
            nc.sync.dma_start(ektv_sb[0:64, p, 0:64],
                              ktv_sb[:, HD * h0:HD * (h0 + 1)])
            nc.sync.dma_start(ektv_sb[64:128, p, 64:128],
                              ktv_sb[:, HD * h1:HD * (h1 + 1)])
        for r in range(6):  # rolled pairs p=6+r: expanded heads 12+2r, 13+2r
            p = 6 + r
            h, h2 = 2 * r, 2 * r + 1
            h3 = (h2 + 1) % HEADS
            nc.sync.dma_start(ektv_sb[0:64, p, 0:32],
                              ktv_sb[:, HD * h + 32:HD * (h + 1)])
            nc.sync.dma_start(ektv_sb[0:64, p, 32:64],
                              ktv_sb[:, HD * h2:HD * h2 + 32])
            nc.sync.dma_start(ektv_sb[64:128, p, 64:96],
                              ktv_sb[:, HD * h2 + 32:HD * (h2 + 1)])
            nc.sync.dma_start(ektv_sb[64:128, p, 96:128],
                              ktv_sb[:, HD * h3:HD * h3 + 32])

        # ---------------- Phase 3: attn + LePE + proj ----------------
        # All taps on DVE, in 3 independent chains grouped by dx so the
        # in-place RAW chains interleave (pipe-drain overlap): dx=0 taps
        # accumulate onto mt (attn already there); dx=+1 onto mtB (seeded by
        # its dy=0 tap, all write x 0:63); dx=-1 onto mtC (x 1:64). Two
        # range-limited merges fold mtB/mtC into mt.
        CHAIN_A = [(0, 0), (-1, 0), (1, 0)]
        CHAIN_B = [(0, 1), (-1, 1), (1, 1)]
        CHAIN_C = [(0, -1), (-1, -1), (1, -1)]
        with tc.tile_pool(name="p3", bufs=2) as p3, \
             tc.tile_pool(name="ps_at", bufs=2, space="PSUM") as ps_at, \
             tc.tile_pool(name="ps_y", bufs=2, space="PSUM") as ps_y:
            for it in range(N // NT3):
                n0 = it * NT3
                rows = NT3 // 64          # image rows in this tile
                y0 = n0 // 64             # first global image row
                # rolled-q stream tile with 64-halo on both sides
                a = max(0, n0 - 64)
                b = min(N, n0 + NT3 + 64)
                off = a - (n0 - 64)
                qtr = p3.tile([128, CC, NT3 + 128], bf16, tag="qtr")
                for t in range(CC):
                    nc.scalar.dma_start(qtr[0:96, t, off:off + (b - a)],
                                        qT[32:128, t, a:b])
                    nc.scalar.dma_start(qtr[96:128, t, off:off + (b - a)],
                                        qT[0:32, (t + 1) % CC, a:b])

                mt = p3.tile([128, EC, NT3], bf16, tag="mt")
                for p in range(EC):
                    mtB = p3.tile([128, NT3], bf16, tag="mtB")
                    mtC = p3.tile([128, NT3], bf16, tag="mtC")
                    for half in range(NT3 // 512):
                        at_ps = ps_at.tile([128, 512], f32, tag="at")
                        if p < 6:
                            rhs = qT[:, p, n0 + 512 * half:n0 + 512 * (half + 1)]
                        else:
                            rhs = qtr[:, p - 6,
                                      64 + 512 * half:64 + 512 * (half + 1)]
                        nc.tensor.matmul(at_ps, ektv_sb[:, p, :], rhs,
                                         start=True, stop=True)
                        nc.scalar.copy(out=mt[:, p, 512 * half:512 * (half + 1)],
                                       in_=at_ps)

                    out3 = {
                        0: mt[:, p, :].rearrange("p (y x) -> p y x", x=64),
                        1: mtB.rearrange("p (y x) -> p y x", x=64),
                        -1: mtC.rearrange("p (y x) -> p y x", x=64),
                    }
                    if p < 6:
                        src3 = qT[:, p, :].rearrange("p (y x) -> p y x", x=64)
                    else:
                        src3 = qtr[:, p - 6, :].rearrange("p (y x) -> p y x", x=64)
                    # interleave the three chains so DVE pipe-drains overlap.
                    # dy=+1 taps: product w*q_shift on the lightly-loaded ACT
                    # engine; DVE folds it in with a 2x-mode tensor_tensor add.
                    for (dy, dx) in [c[i] for i in range(3)
                                     for c in (CHAIN_A, CHAIN_B, CHAIN_C)]:
                        r0 = max(0, -(y0 + dy))
                        r1 = rows - max(0, y0 + rows - 1 + dy - 63)
                        if dx == 1:
                            xo, xi = (0, 63), (1, 64)
                        elif dx == -1:
                            xo, xi = (1, 64), (0, 63)
                        else:
                            xo, xi = (0, 64), (0, 64)
                        if p < 6:
                            s0 = y0 + r0 + dy
                            s1 = y0 + r1 + dy
                        else:
                            s0 = r0 + dy + 1
                            s1 = r1 + dy + 1
                        widx = (dy + 1) * 3 + (dx + 1)
                        w_ap = taps_sb[:, p, widx:widx + 1]
                        i_ap = src3[:, s0:s1, xi[0]:xi[1]]
                        o_ap = out3[dx][:, r0:r1, xo[0]:xo[1]]
                        if dy == 0 and dx != 0:
                            # chain seed: overwrite (full row range for dy=0)
                            nc.vector.tensor_scalar(
                                out=o_ap, in0=i_ap, scalar1=w_ap,
                                scalar2=None, op0=OP.mult)
                        elif dy == 1:
                            nrow = r1 - r0
                            nx = xo[1] - xo[0]
                            tmp = p3.tile([128, rows, 64], bf16, tag="acttmp")
                            t_ap = tmp[:, :nrow, :nx]
                            nc.scalar.activation(out=t_ap, in_=i_ap,
                                                 func=AF.Copy, scale=w_ap)
                            nc.vector.tensor_tensor(o_ap, o_ap, t_ap, OP.add)
                        else:
                            nc.vector.scalar_tensor_tensor(
                                out=o_ap, in0=i_ap, scalar=w_ap,
                                in1=o_ap, op0=OP.mult, op1=OP.add)
                    m3 = mt[:, p, :].rearrange("p (y x) -> p y x", x=64)
                    b3 = mtB.rearrange("p (y x) -> p y x", x=64)
                    c3 = mtC.rearrange("p (y x) -> p y x", x=64)
                    nc.vector.tensor_tensor(
                        m3[:, :, 0:63], m3[:, :, 0:63], b3[:, :, 0:63], OP.add)
                    nc.vector.tensor_tensor(
                        m3[:, :, 1:64], m3[:, :, 1:64], c3[:, :, 1:64], OP.add)

                # proj
                for sub in range(NT3 // 128):
                    y_ps = ps_y.tile([128, DIM], f32, tag="y")
                    for e in range(EC):
                        lhs = mt[:, e, 128 * sub:128 * (sub + 1)]
                        st, sp = (e == 0), (e == EC - 1)
                        nc.tensor.matmul(y_ps[:, 0:512], lhs, wp_sb[:, e, 0:512],
                                         start=st, stop=sp)
                        nc.tensor.matmul(y_ps[:, 512:768], lhs, wp_sb[:, e, 512:768],
                                         start=st, stop=sp)
                    y_sb = p3.tile([128, DIM], bf16, tag="ysb")
                    nc.vector.tensor_tensor(y_sb, y_ps, bias_sb, OP.add)
                    nc.gpsimd.dma_start(
                        y_d[n0 + 128 * sub:n0 + 128 * (sub + 1), :], y_sb)

    nc.compile()
    return nc


def _run(nc, xt_dev):
    """Execute the prebuilt Bass module on B cores via PJRT/shard_map.

    Like bass2jax.run_bass_via_pjrt, but output buffers are created
    device-side (no host->device upload of donated zeros) and the input
    is already device-resident.
    """
    import jax
    import jax.numpy as jnp
    import concourse.mybir as mybir
    from concourse.bass2jax import _bass_exec_p, partition_id_tensor
    from jax.experimental.shard_map import shard_map
    from jax.sharding import Mesh, NamedSharding, PartitionSpec

    partition_name = (
        nc.partition_id_tensor.name if nc.partition_id_tensor else None)
    in_names, out_names, out_avals = [], [], []
    for alloc in nc.m.functions[0].allocations:
        if not isinstance(alloc, mybir.MemoryLocationSet):
            continue
        name = alloc.memorylocations[0].name
        if alloc.kind == "ExternalInput":
            if name != partition_name:
                in_names.append(name)
        elif alloc.kind == "ExternalOutput":
            out_names.append(name)
            out_avals.append(jax.core.ShapedArray(
                tuple(alloc.tensor_shape), mybir.dt.np(alloc.dtype)))
    assert in_names == ["xt"] and out_names == ["y"], (in_names, out_names)
    n_params = len(in_names)
    n_outs = len(out_avals)
    donate = tuple(range(n_params, n_params + n_outs))
    all_names = in_names + out_names
    if partition_name is not None:
        all_names = all_names + [partition_name]

    def _body(*args):
        operands = list(args)
        if partition_name is not None:
            operands.append(partition_id_tensor())
        outs = _bass_exec_p.bind(
            *operands,
            out_avals=tuple(out_avals),
            in_names=tuple(all_names),
            out_names=tuple(out_names),
            lowering_input_output_aliases=(),
            sim_require_finite=True,
            sim_require_nnan=True,
            nc=nc,
        )
        return tuple(outs)

    mesh, shard, pspec = _mesh_shard()
    if _sess.get("exec_nc") is not nc:
        sharded = jax.jit(
            shard_map(_body, mesh=mesh, in_specs=(pspec,) * (n_params + n_outs),
                      out_specs=(pspec,) * n_outs, check_rep=False),
            donate_argnums=donate, keep_unused=True)
        _sess["exec"] = sharded.lower(
            jax.ShapeDtypeStruct((B * DIM, N), xt_dev.dtype),
            *[jax.ShapeDtypeStruct((B * a.shape[0], *a.shape[1:]), a.dtype)
              for a in out_avals],
        ).compile()
        _sess["zeros_jit"] = [
            jax.jit(lambda a=a: jnp.zeros((B * a.shape[0], *a.shape[1:]),
                                          a.dtype), out_shardings=shard)
            for a in out_avals
        ]
        _sess["exec_nc"] = nc
    zeros_dev = [zj() for zj in _sess["zeros_jit"]]
    outs = _sess["exec"](xt_dev, *zeros_dev)
    return np.asarray(outs[0]).reshape(B, N, DIM)


def _mesh_shard():
    import jax
    from jax.experimental.shard_map import shard_map  # noqa: F401
    from jax.sharding import Mesh, NamedSharding, PartitionSpec

    devices = jax.devices()[:B]
    mesh = Mesh(np.asarray(devices), ("core",))
    pspec = PartitionSpec("core")
    return mesh, NamedSharding(mesh, pspec), pspec


_sess = {}


def kernel(x, w_q, w_kv, w_proj, b_proj, w_lepe, b_lepe):
    import ml_dtypes

    _install_cc_cache()

    bf = ml_dtypes.bfloat16
    x = np.asarray(x, np.float32)
    w_q = np.asarray(w_q, np.float32)
    w_kv = np.asarray(w_kv, np.float32)
    w_proj = np.asarray(w_proj, np.float32)
    b_proj = np.asarray(b_proj, np.float32)
    w_lepe = np.asarray(w_lepe, np.float32)
    b_lepe = np.asarray(b_lepe, np.float32)

    consts = {
        "wq": np.ascontiguousarray(w_q).astype(bf),
        "wkv": np.ascontiguousarray(w_kv).astype(bf),
        "wp": np.ascontiguousarray(w_proj).astype(bf),
        "taps": np.ascontiguousarray(w_lepe.reshape(EDIM, 9)).astype(np.float32),
        "bias": np.ascontiguousarray(np.broadcast_to(
            (b_proj.astype(np.float64)
             + b_lepe.astype(np.float64) @ w_proj.astype(np.float64)
             ).astype(np.float32), (128, DIM))),
    }
    key = hashlib.sha256(
        b"|".join(np.ascontiguousarray(v).tobytes() for v in consts.values())
    ).hexdigest()

    # build the Bass module on a worker thread, overlapped with host prep of
    # xt, jax/axon init, and the (bandwidth-bound) input upload
    import threading

    box = {}

    def _builder():
        try:
            if _sess.get("key") != key:
                _sess["nc"] = _build_nc(consts)
                _sess["key"] = key
        except Exception as e:
            box["build_err"] = e

    th = threading.Thread(target=_builder)
    th.start()

    xt = np.ascontiguousarray(x.transpose(0, 2, 1)).astype(bf)  # (B, DIM, N)
    xin = xt.reshape(B * DIM, N)
    xt_dev = xin
    try:
        import jax

        _, shard, _ = _mesh_shard()
        xt_dev = jax.device_put(xin, shard)
        jax.block_until_ready(xt_dev)
    except Exception:
        pass

    th.join()
    if "build_err" in box:
        raise box["build_err"]
    y = _run(_sess["nc"], xt_dev)
    return y.astype(np.float32)


# revision 33
# speedup vs baseline: 2.2249x; 2.0296x over previous
"""nn_Attention_6373731467473 — linear attention w/ head expansion + LePE.

Full-input contract: kernel(**inputs) takes unsharded inputs, returns full
output. Data-parallel over batch: 8 batch elements -> 8 NeuronCores, no
collectives. Per core, everything runs in bf16 matmuls with fp32 PSUM
accumulation (tolerance is 2e-2 relative).

Pipeline per core (batch element b):
  P1: stream xT tiles; qT = (x @ w_q)^T   [qcol, n] resident SBUF;
      k|v joint row-major matmul per 128-row chunk; softmax(k) over
      head_dim (exp on ACT, batched fast reciprocal on DVE);
      ktv[h] = softmax(k)_h^T @ v_h PSUM-accumulated over all n.
  P2: assemble block-diag expanded-ktv lhsT tiles (DMA SBUF->SBUF),
      attention scale 1/sqrt(64) folded in.
  P3: per 1024-col tile: attn^T chunks via block-diag matmuls on qT /
      rolled qT (partition-shifted copy); LePE depthwise 3x3 conv as 9
      per-partition-scaled shifted accumulations, spread across DVE
      (scalar_tensor_tensor chains grouped by dx) and ACT (scaled
      copies folded in with 2x-mode adds); proj matmuls + bias into y.

Host prep: transpose+cast x to bf16 (768, 4096) per batch; weights are
baked into the NEFF as consts (cache-keyed on their bytes), with
b_lepe @ w_proj + b_proj folded into one bias and LePE taps (1536, 9).

Wall-clock notes: the NEFF compile is disk-cached at the libneuronxla
hook level; the Bass build runs on a worker thread overlapped with the
host-side transpose and the sharded input upload; output is bf16 to
halve the device->host transfer.
"""

import hashlib
import os
import pickle
from contextlib import ExitStack

import numpy as np

B, N, DIM = 8, 4096, 768
HEADS, HD = 12, 64
EXP = 2
EDIM = EXP * DIM  # 1536
CC = DIM // 128   # 6 contraction chunks
EC = EDIM // 128  # 12 expanded chunks
NT1 = 512         # phase-1 n-tile
NT3 = 1024        # phase-3 n-tile
SCALE = HD ** -0.5

_CACHE_DIR = os.environ.get("BASS_NEFF_DISK_CACHE", "/root/.cache/bass_neff_cache")


def _install_cc_cache():
    """Disk-cache the HLO->NEFF compile (walrus is the slow step)."""
    try:
        import libneuronxla
        from concourse import bass2jax

        bass2jax.install_neuronx_cc_hook()
        inner = bass2jax.neuronx_cc_hook

        def cached_cc(code, code_format, platform_version, file_prefix):
            try:
                key = hashlib.sha256(
                    b"v1|" + bytes(code) + b"|" + bytes(code_format)
                ).hexdigest()
                path = os.path.join(_CACHE_DIR, key + ".pkl")
                if os.path.exists(path):
                    with open(path, "rb") as f:
                        return pickle.load(f)
            except Exception:
                path = None
            r = inner(code, code_format, platform_version, file_prefix)
            if path is not None:
                try:
                    os.makedirs(_CACHE_DIR, exist_ok=True)
                    tmp = path + f".tmp{os.getpid()}"
                    with open(tmp, "wb") as f:
                        pickle.dump(r, f)
                    os.replace(tmp, path)
                except Exception:
                    pass
            return r

        libneuronxla.neuronx_cc = cached_cc
    except Exception:
        pass


def _build_nc(consts):
    import concourse.bacc as bacc
    import concourse.mybir as mybir
    import concourse.tile as tile

    f32 = mybir.dt.float32
    bf16 = mybir.dt.bfloat16
    AX = mybir.AxisListType
    OP = mybir.AluOpType
    AF = mybir.ActivationFunctionType

    nc = bacc.Bacc("TRN2", target_bir_lowering=False, debug=False, num_devices=B)

    xt_d = nc.dram_tensor("xt", [DIM, N], bf16, kind="ExternalInput").ap()
    wq_d = nc.inline_tensor(consts["wq"], "wq").ap()
    wkv_d = nc.inline_tensor(consts["wkv"], "wkv").ap()
    wp_d = nc.inline_tensor(consts["wp"], "wp").ap()
    taps_d = nc.inline_tensor(consts["taps"], "taps").ap()
    bias_d = nc.inline_tensor(consts["bias"], "bias").ap()
    y_d = nc.dram_tensor("y", [N, DIM], bf16, kind="ExternalOutput").ap()

    with tile.TileContext(nc) as tc, ExitStack() as ctx:
        persist = ctx.enter_context(tc.tile_pool(name="persist", bufs=1))
        qT = persist.tile([128, CC, N], bf16)           # q^T, chunk-major
        wp_sb = persist.tile([128, EC, DIM], bf16)
        taps_sb = persist.tile([128, EC, 9], f32)
        bias_sb = persist.tile([128, DIM], f32)
        ektv_sb = persist.tile([128, EC, 128], bf16)    # block-diag lhsT per pair
        ktv_sb = persist.tile([64, HEADS * HD], bf16)   # scaled bf16 ktv

        nc.gpsimd.dma_start(wp_sb, wp_d.rearrange("(t p) m -> p t m", p=128))
        nc.gpsimd.dma_start(taps_sb, taps_d.rearrange("(t p) s -> p t s", p=128))
        nc.gpsimd.dma_start(bias_sb, bias_d)

        # ---------------- Phase 1: qT, k/v, softmax, ktv ----------------
        with tc.tile_pool(name="p1", bufs=3) as p1, \
             tc.tile_pool(name="p1w", bufs=1) as p1w, \
             tc.tile_pool(name="ps_q", bufs=2, space="PSUM") as ps_q, \
             tc.tile_pool(name="ps_kv", bufs=1, space="PSUM") as ps_kv, \
             tc.tile_pool(name="ps_ktv", bufs=1, space="PSUM") as ps_ktv:
            wq_sb = p1w.tile([128, CC, DIM], bf16)
            wkv_sb = p1w.tile([128, CC, 2 * DIM], bf16)
            nc.scalar.dma_start(wq_sb, wq_d.rearrange("(t p) m -> p t m", p=128))
            nc.sync.dma_start(wkv_sb, wkv_d.rearrange("(t p) m -> p t m", p=128))

            xt_r = xt_d.rearrange("(c p) n -> p c n", p=128)
            # single PSUM accumulator for ktv across the whole n loop; each
            # bank's first matmul carries start=True (clears has_written once)
            ktv_ps = ps_ktv.tile([64, HEADS * HD], f32)
            n_tiles1 = N // NT1
            subs1 = NT1 // 128
            for it in range(n_tiles1):
                n0 = it * NT1
                xt_sb = p1.tile([128, CC, NT1], bf16, tag="xt")
                nc.sync.dma_start(xt_sb, xt_r[:, :, n0:n0 + NT1])

                # qT chunks
                for t in range(CC):
                    q_ps = ps_q.tile([128, NT1], f32, tag="q")
                    for cc in range(CC):
                        nc.tensor.matmul(
                            q_ps, wq_sb[:, cc, 128 * t:128 * (t + 1)],
                            xt_sb[:, cc, :],
                            start=(cc == 0), stop=(cc == CC - 1),
                        )
                    nc.scalar.copy(out=qT[:, t, n0:n0 + NT1], in_=q_ps)

                # k/v rows (joint 1536-wide matmul), softmax, ktv accumulation.
                # Pass 1 computes exp(k)/v and per-head sums for all 4 subs;
                # one batched fast-reciprocal; pass 2 normalizes and runs ktv.
                exp_sb = p1.tile([128, subs1, HEADS, HD], bf16, tag="exp")
                v_bf = p1.tile([128, subs1, DIM], bf16, tag="vb")
                ssum = p1.tile([128, subs1, HEADS], f32, tag="ssum")
                rec = p1.tile([128, subs1, HEADS], f32, tag="rec")
                for sub in range(subs1):
                    kv_ps = ps_kv.tile([128, 2 * DIM], f32, tag="kv")
                    for cc in range(CC):
                        lhs = xt_sb[:, cc, 128 * sub:128 * (sub + 1)]
                        st, sp = (cc == 0), (cc == CC - 1)
                        for blk in range(3):
                            nc.tensor.matmul(
                                kv_ps[:, 512 * blk:512 * (blk + 1)], lhs,
                                wkv_sb[:, cc, 512 * blk:512 * (blk + 1)],
                                start=st, stop=sp)
                    nc.scalar.activation(
                        out=exp_sb[:, sub].rearrange("p h d -> p (h d)"),
                        in_=kv_ps[:, 0:DIM], func=AF.Exp)
                    nc.vector.reduce_sum(ssum[:, sub], exp_sb[:, sub], axis=AX.X)
                    nc.scalar.copy(out=v_bf[:, sub], in_=kv_ps[:, DIM:2 * DIM])
                nc.vector.reciprocal_approx_fast(
                    out=rec.rearrange("p s h -> p (s h)"),
                    in_=ssum.rearrange("p s h -> p (s h)"))
                for sub in range(subs1):
                    ks_bf = p1.tile([128, HEADS, HD], bf16, tag="ks")
                    nc.vector.tensor_tensor(
                        ks_bf, exp_sb[:, sub],
                        rec[:, sub, :, None].broadcast_to([128, HEADS, HD]),
                        OP.mult)
                    first = (it == 0 and sub == 0)
                    last = (it == n_tiles1 - 1 and sub == subs1 - 1)
                    for h in range(HEADS):
                        nc.tensor.matmul(
                            ktv_ps[:, HD * h:HD * (h + 1)],
                            ks_bf[:, h, :], v_bf[:, sub, HD * h:HD * (h + 1)],
                            start=(first and h % 8 == 0),
                            stop=(last and h in (7, 11)),
                            skip_group_check=True,
                        )

            # scale into bf16 (attention scale folded into ektv)
            nc.scalar.mul(out=ktv_sb, in_=ktv_ps, mul=SCALE)

        # ---------------- Phase 2: block-diag expanded ktv ----------------
        nc.vector.memset(ektv_sb, 0.0)
        for p in range(6):  # non-rolled pairs: heads 2p, 2p+1
            h0, h1 = 2 * p, 2 * p + 1
            nc.sync.dma_start(ektv_sb[0:64, p, 0:64],
                              ktv_sb[:, HD * h0:HD * (h0 + 1)])
            nc.sync.dma_start(ektv_sb[64:128, p, 64:128],
                              ktv_sb[:, HD * h1:HD * (h1 + 1)])
        for r in range(6):  # rolled pairs p=6+r: expanded heads 12+2r, 13+2r
            p = 6 + r
            h, h2 = 2 * r, 2 * r + 1
            h3 = (h2 + 1) % HEADS
            nc.sync.dma_start(ektv_sb[0:64, p, 0:32],
                              ktv_sb[:, HD * h + 32:HD * (h + 1)])
            nc.sync.dma_start(ektv_sb[0:64, p, 32:64],
                              ktv_sb[:, HD * h2:HD * h2 + 32])
            nc.sync.dma_start(ektv_sb[64:128, p, 64:96],
                              ktv_sb[:, HD * h2 + 32:HD * (h2 + 1)])
            nc.sync.dma_start(ektv_sb[64:128, p, 96:128],
                              ktv_sb[:, HD * h3:HD * h3 + 32])

        # ---------------- Phase 3: attn + LePE + proj ----------------
        # All taps on DVE, in 3 independent chains grouped by dx so the
        # in-place RAW chains interleave (pipe-drain overlap): dx=0 taps
        # accumulate onto mt (attn already there); dx=+1 onto mtB (seeded by
        # its dy=0 tap, all write x 0:63); dx=-1 onto mtC (x 1:64). Two
        # range-limited merges fold mtB/mtC into mt.
        CHAIN_A = [(0, 0), (-1, 0), (1, 0)]
        CHAIN_B = [(0, 1), (-1, 1), (1, 1)]
        CHAIN_C = [(0, -1), (-1, -1), (1, -1)]
        with tc.tile_pool(name="p3", bufs=2) as p3, \
             tc.tile_pool(name="ps_at", bufs=2, space="PSUM") as ps_at, \
             tc.tile_pool(name="ps_y", bufs=2, space="PSUM") as ps_y:
            for it in range(N // NT3):
                n0 = it * NT3
                rows = NT3 // 64          # image rows in this tile
                y0 = n0 // 64             # first global image row
                # rolled-q stream tile with 64-halo on both sides
                a = max(0, n0 - 64)
                b = min(N, n0 + NT3 + 64)
                off = a - (n0 - 64)
                qtr = p3.tile([128, CC, NT3 + 128], bf16, tag="qtr")
                for t in range(CC):
                    nc.scalar.dma_start(qtr[0:96, t, off:off + (b - a)],
                                        qT[32:128, t, a:b])
                    nc.scalar.dma_start(qtr[96:128, t, off:off + (b - a)],
                                        qT[0:32, (t + 1) % CC, a:b])

                mt = p3.tile([128, EC, NT3], bf16, tag="mt")
                for p in range(EC):
                    mtB = p3.tile([128, NT3], bf16, tag="mtB")
                    mtC = p3.tile([128, NT3], bf16, tag="mtC")
                    for half in range(NT3 // 512):
                        at_ps = ps_at.tile([128, 512], f32, tag="at")
                        if p < 6:
                            rhs = qT[:, p, n0 + 512 * half:n0 + 512 * (half + 1)]
                        else:
                            rhs = qtr[:, p - 6,
                                      64 + 512 * half:64 + 512 * (half + 1)]
                        nc.tensor.matmul(at_ps, ektv_sb[:, p, :], rhs,
                                         start=True, stop=True)
                        nc.scalar.copy(out=mt[:, p, 512 * half:512 * (half + 1)],
                                       in_=at_ps)

                    out3 = {
                        0: mt[:, p, :].rearrange("p (y x) -> p y x", x=64),
                        1: mtB.rearrange("p (y x) -> p y x", x=64),
                        -1: mtC.rearrange("p (y x) -> p y x", x=64),
                    }
                    if p < 6:
                        src3 = qT[:, p, :].rearrange("p (y x) -> p y x", x=64)
                    else:
                        src3 = qtr[:, p - 6, :].rearrange("p (y x) -> p y x", x=64)
                    # interleave the three chains so DVE pipe-drains overlap.
                    # dy=+1 taps: product w*q_shift on the lightly-loaded ACT
                    # engine; DVE folds it in with a 2x-mode tensor_tensor add.
                    for (dy, dx) in [c[i] for i in range(3)
                                     for c in (CHAIN_A, CHAIN_B, CHAIN_C)]:
                        r0 = max(0, -(y0 + dy))
                        r1 = rows - max(0, y0 + rows - 1 + dy - 63)
                        if dx == 1:
                            xo, xi = (0, 63), (1, 64)
                        elif dx == -1:
                            xo, xi = (1, 64), (0, 63)
                        else:
                            xo, xi = (0, 64), (0, 64)
                        if p < 6:
                            s0 = y0 + r0 + dy
                            s1 = y0 + r1 + dy
                        else:
                            s0 = r0 + dy + 1
                            s1 = r1 + dy + 1
                        widx = (dy + 1) * 3 + (dx + 1)
                        w_ap = taps_sb[:, p, widx:widx + 1]
                        i_ap = src3[:, s0:s1, xi[0]:xi[1]]
                        o_ap = out3[dx][:, r0:r1, xo[0]:xo[1]]
                        if dy == 0 and dx != 0:
                            # chain seed: overwrite (full row range for dy=0),
                            # scaled copy on ACT
                            nc.scalar.activation(out=o_ap, in_=i_ap,
                                                 func=AF.Copy, scale=w_ap)
                        elif dy == 1:
                            nrow = r1 - r0
                            nx = xo[1] - xo[0]
                            tmp = p3.tile([128, rows, 64], bf16, tag="acttmp")
                            t_ap = tmp[:, :nrow, :nx]
                            nc.scalar.activation(out=t_ap, in_=i_ap,
                                                 func=AF.Copy, scale=w_ap)
                            nc.vector.tensor_tensor(o_ap, o_ap, t_ap, OP.add)
                        else:
                            nc.vector.scalar_tensor_tensor(
                                out=o_ap, in0=i_ap, scalar=w_ap,
                                in1=o_ap, op0=OP.mult, op1=OP.add)
                    m3 = mt[:, p, :].rearrange("p (y x) -> p y x", x=64)
                    b3 = mtB.rearrange("p (y x) -> p y x", x=64)
                    c3 = mtC.rearrange("p (y x) -> p y x", x=64)
                    nc.vector.tensor_tensor(
                        m3[:, :, 0:63], m3[:, :, 0:63], b3[:, :, 0:63], OP.add)
                    nc.vector.tensor_tensor(
                        m3[:, :, 1:64], m3[:, :, 1:64], c3[:, :, 1:64], OP.add)

                # proj
                for sub in range(NT3 // 128):
                    y_ps = ps_y.tile([128, DIM], f32, tag="y")
                    for e in range(EC):
                        lhs = mt[:, e, 128 * sub:128 * (sub + 1)]
                        st, sp = (e == 0), (e == EC - 1)
                        nc.tensor.matmul(y_ps[:, 0:512], lhs, wp_sb[:, e, 0:512],
                                         start=st, stop=sp)
                        nc.tensor.matmul(y_ps[:, 512:768], lhs, wp_sb[:, e, 512:768],
                                         start=st, stop=sp)
                    y_sb = p3.tile([128, DIM], bf16, tag="ysb")
                    nc.vector.tensor_tensor(y_sb, y_ps, bias_sb, OP.add)
                    nc.gpsimd.dma_start(
                        y_d[n0 + 128 * sub:n0 + 128 * (sub + 1), :], y_sb)

    nc.compile()
    return nc


def _run(nc, xt_dev):
    """Execute the prebuilt Bass module on B cores via PJRT/shard_map.

    Like bass2jax.run_bass_via_pjrt, but output buffers are created
    device-side (no host->device upload of donated zeros) and the input
    is already device-resident.
    """
    import jax
    import jax.numpy as jnp
    import concourse.mybir as mybir
    from concourse.bass2jax import _bass_exec_p, partition_id_tensor
    from jax.experimental.shard_map import shard_map
    from jax.sharding import Mesh, NamedSharding, PartitionSpec

    partition_name = (
        nc.partition_id_tensor.name if nc.partition_id_tensor else None)
    in_names, out_names, out_avals = [], [], []
    for alloc in nc.m.functions[0].allocations:
        if not isinstance(alloc, mybir.MemoryLocationSet):
            continue
        name = alloc.memorylocations[0].name
        if alloc.kind == "ExternalInput":
            if name != partition_name:
                in_names.append(name)
        elif alloc.kind == "ExternalOutput":
            out_names.append(name)
            out_avals.append(jax.core.ShapedArray(
                tuple(alloc.tensor_shape), mybir.dt.np(alloc.dtype)))
    assert in_names == ["xt"] and out_names == ["y"], (in_names, out_names)
    n_params = len(in_names)
    n_outs = len(out_avals)
    donate = tuple(range(n_params, n_params + n_outs))
    all_names = in_names + out_names
    if partition_name is not None:
        all_names = all_names + [partition_name]

    def _body(*args):
        operands = list(args)
        if partition_name is not None:
            operands.append(partition_id_tensor())
        outs = _bass_exec_p.bind(
            *operands,
            out_avals=tuple(out_avals),
            in_names=tuple(all_names),
            out_names=tuple(out_names),
            lowering_input_output_aliases=(),
            sim_require_finite=True,
            sim_require_nnan=True,
            nc=nc,
        )
        return tuple(outs)

    mesh, shard, pspec = _mesh_shard()
    if _sess.get("exec_nc") is not nc:
        sharded = jax.jit(
            shard_map(_body, mesh=mesh, in_specs=(pspec,) * (n_params + n_outs),
                      out_specs=(pspec,) * n_outs, check_rep=False),
            donate_argnums=donate, keep_unused=True)
        _sess["exec"] = sharded.lower(
            jax.ShapeDtypeStruct((B * DIM, N), xt_dev.dtype),
            *[jax.ShapeDtypeStruct((B * a.shape[0], *a.shape[1:]), a.dtype)
              for a in out_avals],
        ).compile()
        _sess["zeros_jit"] = [
            jax.jit(lambda a=a: jnp.zeros((B * a.shape[0], *a.shape[1:]),
                                          a.dtype), out_shardings=shard)
            for a in out_avals
        ]
        _sess["exec_nc"] = nc
    zeros_dev = [zj() for zj in _sess["zeros_jit"]]
    outs = _sess["exec"](xt_dev, *zeros_dev)
    return np.asarray(outs[0]).reshape(B, N, DIM)


def _mesh_shard():
    import jax
    from jax.experimental.shard_map import shard_map  # noqa: F401
    from jax.sharding import Mesh, NamedSharding, PartitionSpec

    devices = jax.devices()[:B]
    mesh = Mesh(np.asarray(devices), ("core",))
    pspec = PartitionSpec("core")
    return mesh, NamedSharding(mesh, pspec), pspec


_sess = {}


def kernel(x, w_q, w_kv, w_proj, b_proj, w_lepe, b_lepe):
    import ml_dtypes

    _install_cc_cache()

    bf = ml_dtypes.bfloat16
    x = np.asarray(x, np.float32)
    w_q = np.asarray(w_q, np.float32)
    w_kv = np.asarray(w_kv, np.float32)
    w_proj = np.asarray(w_proj, np.float32)
    b_proj = np.asarray(b_proj, np.float32)
    w_lepe = np.asarray(w_lepe, np.float32)
    b_lepe = np.asarray(b_lepe, np.float32)

    consts = {
        "wq": np.ascontiguousarray(w_q).astype(bf),
        "wkv": np.ascontiguousarray(w_kv).astype(bf),
        "wp": np.ascontiguousarray(w_proj).astype(bf),
        "taps": np.ascontiguousarray(w_lepe.reshape(EDIM, 9)).astype(np.float32),
        "bias": np.ascontiguousarray(np.broadcast_to(
            (b_proj.astype(np.float64)
             + b_lepe.astype(np.float64) @ w_proj.astype(np.float64)
             ).astype(np.float32), (128, DIM))),
    }
    key = hashlib.sha256(
        b"|".join(np.ascontiguousarray(v).tobytes() for v in consts.values())
    ).hexdigest()

    # build the Bass module on a worker thread, overlapped with host prep of
    # xt, jax/axon init, and the (bandwidth-bound) input upload
    import threading

    box = {}

    def _builder():
        try:
            if _sess.get("key") != key:
                _sess["nc"] = _build_nc(consts)
                _sess["key"] = key
        except Exception as e:
            box["build_err"] = e

    th = threading.Thread(target=_builder)
    th.start()

    xt = np.ascontiguousarray(x.transpose(0, 2, 1)).astype(bf)  # (B, DIM, N)
    xin = xt.reshape(B * DIM, N)
    xt_dev = xin
    try:
        import jax

        _, shard, _ = _mesh_shard()
        xt_dev = jax.device_put(xin, shard)
        jax.block_until_ready(xt_dev)
    except Exception:
        pass

    th.join()
    if "build_err" in box:
        raise box["build_err"]
    y = _run(_sess["nc"], xt_dev)
    return y.astype(np.float32)
